# revision 7
# baseline (speedup 1.0000x reference)
"""GQA attention (B=2,T=2048,D=2048,H=16,KV=4,HD=128, causal+RoPE) on 8 trn2 cores.

Sharding: 4-way head tensor-parallel x 2-way batch data-parallel.
Core c: batch b=c//4, TP shard s=c%4 -> q heads [4s..4s+3], kv head s.

Transpose-free design (v1): scores are computed directly in kv-major layout
per 128-token kv block:  ST[kv,q] = kT_block^T @ qT_stripe  (PE), so
exp(ST) written to SBUF *is* the P^T operand needed by the PV matmul
O^T[hd,q] = V_block^T @ P^T.  The softmax denominator l[q] = colsum(P^T)
comes from a ones-vector matmul ([128,1] lhsT), its reciprocal is
broadcast to all partitions with a K=1 outer-product matmul, and the
normalization is fused into the PSUM->SBUF move of O^T on the DVE.
Causal masking is multiplicative post-exp (binary bf16 tiles) on the 4
diagonal blocks of each 512-wide q stripe.  No PE transposes anywhere,
so the PE stream is dense back-to-back matmuls and the HAM clock gate
stays at 2.4 GHz.
"""

import math
import os
import numpy as np

try:
    import concourse.bass as bass
except ImportError:  # pragma: no cover
    import sys

    sys.path.insert(0, "/opt/trn_rl_repo")
    import concourse.bass as bass

import concourse.mybir as mybir
import concourse.bacc as bacc
from concourse import bass_utils
from concourse.tile import TileContext
from contextlib import ExitStack
from ml_dtypes import bfloat16

B, T, D = 2, 2048, 2048
H, KV, HD = 16, 4, 128
TP = 4  # head-TP ways
NH = H // TP  # q heads per core = 4
NKB = D // 128  # 16 contraction blocks
NTC = T // 512  # 4 token chunks / q stripes
NTB = T // 128  # 16 token blocks
SCALE = 1.0 / math.sqrt(HD)
F32 = mybir.dt.float32
BF16 = mybir.dt.bfloat16
EXP = mybir.ActivationFunctionType.Exp

_program = None
_last_results = None
last_exec_time_ns = None


def _build_program():
    global _program
    if _program is not None:
        return _program

    nc = bacc.Bacc(
        "TRN2",
        target_bir_lowering=False,
        debug=False,
        enable_asserts=False,
        num_devices=8,
    )
    # host-packed layouts: [128 partitions, ...] with j = D/128 contraction blocks
    xp_d = nc.dram_tensor("xp", [128, NKB, T], BF16, kind="ExternalInput").ap()
    wq_d = nc.dram_tensor("Wq", [128, NKB, NH * 128], BF16, kind="ExternalInput").ap()
    wk_d = nc.dram_tensor("Wk", [128, NKB, 128], BF16, kind="ExternalInput").ap()
    wv_d = nc.dram_tensor("Wv", [128, NKB, 128], BF16, kind="ExternalInput").ap()
    wo_d = nc.dram_tensor("Wo", [128, NH, D], BF16, kind="ExternalInput").ap()
    cq_d = nc.dram_tensor("cosq", [128, T], F32, kind="ExternalInput").ap()
    sq_d = nc.dram_tensor("sinq", [128, T], F32, kind="ExternalInput").ap()
    ck_d = nc.dram_tensor("cosk", [128, T], F32, kind="ExternalInput").ap()
    sk_d = nc.dram_tensor("sink", [128, T], F32, kind="ExternalInput").ap()
    mk_d = nc.dram_tensor("maskv", [128, 4, 512], BF16, kind="ExternalInput").ap()
    y_d = nc.dram_tensor("y", [T, D], BF16, kind="ExternalOutput").ap()

    with TileContext(nc) as tc, ExitStack() as ctx:
        big = ctx.enter_context(tc.tile_pool(name="big", bufs=1))
        xpool = ctx.enter_context(tc.tile_pool(name="xpool", bufs=2))
        ps = ctx.enter_context(tc.tile_pool(name="ps", bufs=3, space="PSUM"))
        ps_o = ctx.enter_context(tc.tile_pool(name="ps_o", bufs=2, space="PSUM"))
        ps_l = ctx.enter_context(tc.tile_pool(name="ps_l", bufs=1, space="PSUM"))
        ps_y = ctx.enter_context(tc.tile_pool(name="ps_y", bufs=2, space="PSUM"))
        rtmp = ctx.enter_context(tc.tile_pool(name="rtmp", bufs=3))
        ptpool = ctx.enter_context(tc.tile_pool(name="ptpool", bufs=2))
        otpool = ctx.enter_context(tc.tile_pool(name="otpool", bufs=2))
        rlpool = ctx.enter_context(tc.tile_pool(name="rlpool", bufs=2))
        bcpool = ctx.enter_context(tc.tile_pool(name="bcpool", bufs=2))
        ypool = ctx.enter_context(tc.tile_pool(name="ypool", bufs=2))

        wq = big.tile([128, NKB, NH * 128], BF16, tag="wq")
        wk = big.tile([128, NKB, 128], BF16, tag="wk")
        wv = big.tile([128, NKB, 128], BF16, tag="wv")
        wo = big.tile([128, NH, D], BF16, tag="wo")
        cq = big.tile([128, T], F32, tag="cq")
        sq = big.tile([128, T], F32, tag="sq")
        ck = big.tile([128, T], F32, tag="ck")
        sk = big.tile([128, T], F32, tag="sk")
        maskv = big.tile([128, 4, 512], BF16, tag="maskv")
        onesc = big.tile([128, 1], BF16, tag="onesc")
        onesr = big.tile([1, 128], BF16, tag="onesr")
        qT = big.tile([128, NH, T], BF16, tag="qT")
        kT = big.tile([128, T], BF16, tag="kT")
        V = big.tile([128, NTB, 128], BF16, tag="V")

        # ---- loads (ordered so early compute unblocks fast) ----
        nc.sync.dma_start(out=wk[:], in_=wk_d[:])
        nc.sync.dma_start(out=ck[:, :1024], in_=ck_d[:, :1024])
        nc.sync.dma_start(out=ck[:, 1024:], in_=ck_d[:, 1024:])
        nc.sync.dma_start(out=sk[:, :1024], in_=sk_d[:, :1024])
        nc.sync.dma_start(out=sk[:, 1024:], in_=sk_d[:, 1024:])
        nc.sync.dma_start(out=wq[:], in_=wq_d[:])
        nc.sync.dma_start(out=cq[:, :1024], in_=cq_d[:, :1024])
        nc.sync.dma_start(out=cq[:, 1024:], in_=cq_d[:, 1024:])
        nc.sync.dma_start(out=sq[:, :1024], in_=sq_d[:, :1024])
        nc.sync.dma_start(out=sq[:, 1024:], in_=sq_d[:, 1024:])
        nc.sync.dma_start(out=wv[:], in_=wv_d[:])
        nc.sync.dma_start(out=maskv[:], in_=mk_d[:])
        nc.sync.dma_start(out=wo[:], in_=wo_d[:])
        nc.vector.memset(onesc[:], 1.0)
        nc.vector.memset(onesr[:], 1.0)

        # ---- projections with fused RoPE, chunk-major over tokens ----
        def rope(pst, cos_sb, sin_sb, dst, sl):
            t1 = rtmp.tile([128, 512], F32, tag="t1")
            nc.vector.tensor_mul(t1[:], pst[:], cos_sb[:, sl])
            t2 = rtmp.tile([128, 512], F32, tag="t2")
            nc.vector.tensor_mul(t2[0:64, :], pst[64:128, :], sin_sb[0:64, sl])
            nc.vector.tensor_mul(t2[64:128, :], pst[0:64, :], sin_sb[64:128, sl])
            nc.vector.tensor_add(dst, t1[:], t2[:])

        for c in range(NTC):
            sl = slice(c * 512, (c + 1) * 512)
            xc = xpool.tile([128, NKB, 512], BF16, tag="xc")
            nc.sync.dma_start(out=xc[:], in_=xp_d[:, :, sl])
            # kT chunk
            pst = ps.tile([128, 512], F32, tag="ps")
            for j in range(NKB):
                nc.tensor.matmul(
                    pst[:],
                    lhsT=wk[:, j, :],
                    rhs=xc[:, j, :],
                    start=(j == 0),
                    stop=(j == NKB - 1),
                )
            rope(pst, ck, sk, kT[:, sl], sl)
            # qT chunks (4 heads)
            for h in range(NH):
                pst = ps.tile([128, 512], F32, tag="ps")
                for j in range(NKB):
                    nc.tensor.matmul(
                        pst[:],
                        lhsT=wq[:, j, h * 128 : (h + 1) * 128],
                        rhs=xc[:, j, :],
                        start=(j == 0),
                        stop=(j == NKB - 1),
                    )
                rope(pst, cq, sq, qT[:, h, sl], sl)
            # V blocks (tokens on partitions)
            for tb in range(4):
                pv = ps.tile([128, 512], F32, tag="ps")
                for j in range(NKB):
                    nc.tensor.matmul(
                        pv[:, :128],
                        lhsT=xc[:, j, tb * 128 : (tb + 1) * 128],
                        rhs=wv[:, j, :],
                        start=(j == 0),
                        stop=(j == NKB - 1),
                    )
                nc.scalar.copy(V[:, c * 4 + tb, :], pv[:, :128])

        # ---- attention + output projection, per q stripe ----
        for s in range(NTC):
            qsl = slice(s * 512, (s + 1) * 512)
            nb = 4 * (s + 1)
            OTs = otpool.tile([128, NH, 512], BF16, tag="OT")
            for h in range(NH):
                PT = ptpool.tile([128, NTB, 512], BF16, tag="PT")
                lp = ps_l.tile([1, 512], F32, tag="lp")
                op = ps_o.tile([128, 512], F32, tag="op")

                def lpv(b):
                    nc.tensor.matmul(
                        lp[:],
                        lhsT=onesc[:],
                        rhs=PT[:, b, :],
                        start=(b == 0),
                        stop=(b == nb - 1),
                    )
                    nc.tensor.matmul(
                        op[:],
                        lhsT=V[:, b, :],
                        rhs=PT[:, b, :],
                        start=(b == 0),
                        stop=(b == nb - 1),
                    )

                for b in range(nb):
                    stp = ps.tile([128, 512], F32, tag="ps")
                    nc.tensor.matmul(
                        stp[:],
                        lhsT=kT[:, b * 128 : (b + 1) * 128],
                        rhs=qT[:, h, qsl],
                        start=True,
                        stop=True,
                    )
                    nc.scalar.activation(PT[:, b, :], stp[:], EXP)
                    if b >= 4 * s:
                        nc.vector.tensor_mul(
                            PT[:, b, :], PT[:, b, :], maskv[:, b - 4 * s, :]
                        )
                    if b >= 2:
                        lpv(b - 2)
                lpv(nb - 2)
                lpv(nb - 1)

                rl = rlpool.tile([1, 512], BF16, tag="rl")
                with nc.allow_low_precision(reason="softmax 1/l in bf16"):
                    nc.vector.reciprocal(rl[:], lp[:])
                bc = ps.tile([128, 512], F32, tag="ps")
                nc.tensor.matmul(bc[:], lhsT=onesr[:], rhs=rl[:], start=True, stop=True)
                bcs = bcpool.tile([128, 512], F32, tag="bcs")
                nc.scalar.copy(bcs[:], bc[:])
                nc.vector.tensor_mul(OTs[:, h, :], op[:], bcs[:])

            # Wo: y[q, :] = sum_h O_h[q, :] @ Wo_h
            for qb in range(4):
                ysb = ypool.tile([128, D], BF16, tag="y")
                for dc in range(4):
                    yp = ps_y.tile([128, 512], F32, tag="yp")
                    for h in range(NH):
                        nc.tensor.matmul(
                            yp[:],
                            lhsT=OTs[:, h, qb * 128 : (qb + 1) * 128],
                            rhs=wo[:, h, dc * 512 : (dc + 1) * 512],
                            start=(h == 0),
                            stop=(h == NH - 1),
                        )
                    nc.scalar.copy(ysb[:, dc * 512 : (dc + 1) * 512], yp[:])
                nc.sync.dma_start(
                    out=y_d[s * 512 + qb * 128 : s * 512 + (qb + 1) * 128, :],
                    in_=ysb[:],
                )

    nc.compile()
    _program = nc
    return nc


def _host_prep(x, Wq, Wk, Wv, Wo):
    x = np.asarray(x, dtype=np.float32)
    Wq = np.asarray(Wq, dtype=np.float32)
    Wk = np.asarray(Wk, dtype=np.float32)
    Wv = np.asarray(Wv, dtype=np.float32)
    Wo = np.asarray(Wo, dtype=np.float32)

    # RoPE even/odd gather folded into weight column permutation (per head)
    perm128 = np.r_[np.arange(0, 128, 2), np.arange(1, 128, 2)]
    permq = np.concatenate([hb * 128 + perm128 for hb in range(H)])
    permk = np.concatenate([hb * 128 + perm128 for hb in range(KV)])
    Wq_p = Wq[:, permq]
    Wk_p = Wk[:, permk]

    pos = np.arange(T, dtype=np.float64)
    inv_freq = 1.0 / (10000.0 ** (np.arange(0, HD, 2, dtype=np.float64) / HD))
    ang = np.einsum("t,f->tf", pos, inv_freq)  # [T, 64]
    cos = np.cos(ang).T.astype(np.float32)  # [64, T]
    sin = np.sin(ang).T.astype(np.float32)
    cosk = np.concatenate([cos, cos], axis=0)  # [128, T]
    sink = np.concatenate([-sin, sin], axis=0)
    cosq = (cosk * SCALE).astype(np.float32)
    sinq = (sink * SCALE).astype(np.float32)

    # binary causal masks for the 4 diagonal blocks of a 512-wide q stripe:
    # keep (kv_l <= q_l - 128*r) for relative kv block r
    kv_l = np.arange(128)[:, None]
    q_l = np.arange(512)[None, :]
    maskv = np.ascontiguousarray(
        np.stack([(kv_l <= q_l - 128 * r) for r in range(4)], axis=1)
    ).astype(bfloat16)  # [128, 4, 512]

    def pack_pj(w, cols):
        # [D, cols] -> [128, NKB, cols] with [p, j, c] = w[j*128+p, c]
        return np.ascontiguousarray(
            w.reshape(NKB, 128, cols).transpose(1, 0, 2)
        ).astype(bfloat16)

    in_maps = []
    for c in range(8):
        b, s = c // 4, c % 4
        xb = np.ascontiguousarray(x[b].T)  # [D, T]
        wo_sh = Wo[s * 512 : (s + 1) * 512, :]  # [512, D]
        in_maps.append(
            {
                "xp": pack_pj(xb, T),
                "Wq": pack_pj(Wq_p[:, s * 512 : (s + 1) * 512], NH * 128),
                "Wk": pack_pj(Wk_p[:, s * 128 : (s + 1) * 128], 128),
                "Wv": pack_pj(Wv[:, s * 128 : (s + 1) * 128], 128),
                "Wo": np.ascontiguousarray(
                    wo_sh.reshape(NH, 128, D).transpose(1, 0, 2)
                ).astype(bfloat16),
                "cosq": cosq,
                "sinq": sinq,
                "cosk": cosk,
                "sink": sink,
                "maskv": maskv,
            }
        )
    return in_maps


def _ensure_ntff_hook():
    """The agent image's antenv lacks axon_hooks, so boot() skips installing
    the NTFF profile hook. Recreate the module and install the hook."""
    import sys
    import types

    try:
        from antenv.axon_hooks import get_axon_ntff_profile_hook  # noqa: F401

        return True
    except ImportError:
        pass
    try:
        import antenv
        from trn_agent_boot.trn_boot import _ntff_profile_via_ctypes

        hook = _ntff_profile_via_ctypes("/opt/axon/libaxon_pjrt.so")
        if hook is None:
            return False
        mod = types.ModuleType("antenv.axon_hooks")
        mod._hook = hook
        mod.set_axon_ntff_profile_hook = lambda h: setattr(mod, "_hook", h)
        mod.get_axon_ntff_profile_hook = lambda: mod._hook
        sys.modules["antenv.axon_hooks"] = mod
        antenv.axon_hooks = mod
        bass_utils.upload_artifacts = lambda d: d
        return True
    except Exception:
        return False


def kernel(x, Wq, Wk, Wv, Wo):
    global _last_results, last_exec_time_ns
    nc = _build_program()
    in_maps = _host_prep(x, Wq, Wk, Wv, Wo)
    trace = bool(int(os.environ.get("KERNEL_TRACE", "0")))
    tmpdir = None
    if trace:
        trace = _ensure_ntff_hook()
        if trace:
            tmpdir = os.environ.get("KERNEL_TRACE_DIR") or None
    res = bass_utils.run_bass_kernel_spmd(
        nc, in_maps, core_ids=list(range(8)), trace=trace, tmpdir=tmpdir
    )
    _last_results = res
    last_exec_time_ns = res.exec_time_ns
    out = np.empty((B, T, D), dtype=np.float32)
    for b in range(B):
        out[b] = sum(
            res.results[4 * b + s]["y"].astype(np.float32) for s in range(TP)
        )
    return out


# revision 8
# speedup vs baseline: 1.2926x; 1.2926x over previous
"""GQA attention (B=2,T=2048,D=2048,H=16,KV=4,HD=128, causal+RoPE) on 8 trn2 cores.

Sharding: 4-way head tensor-parallel x 2-way batch data-parallel.
Core c: batch b=c//4, TP shard s=c%4 -> q heads [4s..4s+3], kv head s.

Transpose-free design (v1): scores are computed directly in kv-major layout
per 128-token kv block:  ST[kv,q] = kT_block^T @ qT_stripe  (PE), so
exp(ST) written to SBUF *is* the P^T operand needed by the PV matmul
O^T[hd,q] = V_block^T @ P^T.  The softmax denominator l[q] = colsum(P^T)
comes from a ones-vector matmul ([128,1] lhsT), its reciprocal is
broadcast to all partitions with a K=1 outer-product matmul, and the
normalization is fused into the PSUM->SBUF move of O^T on the DVE.
Causal masking is multiplicative post-exp (binary bf16 tiles) on the 4
diagonal blocks of each 512-wide q stripe.  No PE transposes anywhere,
so the PE stream is dense back-to-back matmuls and the HAM clock gate
stays at 2.4 GHz.
"""

import math
import os
import numpy as np

try:
    import concourse.bass as bass
except ImportError:  # pragma: no cover
    import sys

    sys.path.insert(0, "/opt/trn_rl_repo")
    import concourse.bass as bass

import concourse.mybir as mybir
import concourse.bacc as bacc
from concourse import bass_utils
from concourse.tile import TileContext
from contextlib import ExitStack
from ml_dtypes import bfloat16

B, T, D = 2, 2048, 2048
H, KV, HD = 16, 4, 128
TP = 4  # head-TP ways
NH = H // TP  # q heads per core = 4
NKB = D // 128  # 16 contraction blocks
NTC = T // 512  # 4 token chunks / q stripes
NTB = T // 128  # 16 token blocks
SCALE = 1.0 / math.sqrt(HD)
F32 = mybir.dt.float32
BF16 = mybir.dt.bfloat16
EXP = mybir.ActivationFunctionType.Exp

_program = None
_last_results = None
last_exec_time_ns = None


def _build_program():
    global _program
    if _program is not None:
        return _program

    nc = bacc.Bacc(
        "TRN2",
        target_bir_lowering=False,
        debug=False,
        enable_asserts=False,
        num_devices=8,
    )
    # host-packed layouts: [128 partitions, ...] with j = D/128 contraction blocks
    xp_d = nc.dram_tensor("xp", [128, NKB, T], BF16, kind="ExternalInput").ap()
    wq_d = nc.dram_tensor("Wq", [128, NKB, NH * 128], BF16, kind="ExternalInput").ap()
    wk_d = nc.dram_tensor("Wk", [128, NKB, 128], BF16, kind="ExternalInput").ap()
    wv_d = nc.dram_tensor("Wv", [128, NKB, 128], BF16, kind="ExternalInput").ap()
    wo_d = nc.dram_tensor("Wo", [128, NH, D], BF16, kind="ExternalInput").ap()
    cq_d = nc.dram_tensor("cosq", [128, T], F32, kind="ExternalInput").ap()
    sq_d = nc.dram_tensor("sinq", [128, T], F32, kind="ExternalInput").ap()
    ck_d = nc.dram_tensor("cosk", [128, T], F32, kind="ExternalInput").ap()
    sk_d = nc.dram_tensor("sink", [128, T], F32, kind="ExternalInput").ap()
    mk_d = nc.dram_tensor("maskv", [128, 4, 512], BF16, kind="ExternalInput").ap()
    y_d = nc.dram_tensor("y", [T, D], BF16, kind="ExternalOutput").ap()

    with TileContext(nc) as tc, ExitStack() as ctx:
        big = ctx.enter_context(tc.tile_pool(name="big", bufs=1))
        xpool = ctx.enter_context(tc.tile_pool(name="xpool", bufs=2))
        ps = ctx.enter_context(tc.tile_pool(name="ps", bufs=3, space="PSUM"))
        ps_o = ctx.enter_context(tc.tile_pool(name="ps_o", bufs=2, space="PSUM"))
        ps_l = ctx.enter_context(tc.tile_pool(name="ps_l", bufs=1, space="PSUM"))
        ps_y = ctx.enter_context(tc.tile_pool(name="ps_y", bufs=2, space="PSUM"))
        rtmp = ctx.enter_context(tc.tile_pool(name="rtmp", bufs=3))
        ptpool = ctx.enter_context(tc.tile_pool(name="ptpool", bufs=2))
        otpool = ctx.enter_context(tc.tile_pool(name="otpool", bufs=2))
        rlpool = ctx.enter_context(tc.tile_pool(name="rlpool", bufs=2))
        bcpool = ctx.enter_context(tc.tile_pool(name="bcpool", bufs=2))
        ypool = ctx.enter_context(tc.tile_pool(name="ypool", bufs=2))

        wq = big.tile([128, NKB, NH * 128], BF16, tag="wq")
        wk = big.tile([128, NKB, 128], BF16, tag="wk")
        wv = big.tile([128, NKB, 128], BF16, tag="wv")
        wo = big.tile([128, NH, D], BF16, tag="wo")
        cq = big.tile([128, T], F32, tag="cq")
        sq = big.tile([128, T], F32, tag="sq")
        ck = big.tile([128, T], F32, tag="ck")
        sk = big.tile([128, T], F32, tag="sk")
        maskv = big.tile([128, 4, 512], BF16, tag="maskv")
        onesc = big.tile([128, 1], BF16, tag="onesc")
        onesr = big.tile([1, 128], BF16, tag="onesr")
        qT = big.tile([128, NH, T], BF16, tag="qT")
        kT = big.tile([128, T], BF16, tag="kT")
        V = big.tile([128, NTB, 128], BF16, tag="V")

        # ---- loads (ordered so early compute unblocks fast) ----
        nc.sync.dma_start(out=wk[:], in_=wk_d[:])
        nc.sync.dma_start(out=ck[:, :1024], in_=ck_d[:, :1024])
        nc.sync.dma_start(out=ck[:, 1024:], in_=ck_d[:, 1024:])
        nc.sync.dma_start(out=sk[:, :1024], in_=sk_d[:, :1024])
        nc.sync.dma_start(out=sk[:, 1024:], in_=sk_d[:, 1024:])
        nc.sync.dma_start(out=wq[:], in_=wq_d[:])
        nc.sync.dma_start(out=cq[:, :1024], in_=cq_d[:, :1024])
        nc.sync.dma_start(out=cq[:, 1024:], in_=cq_d[:, 1024:])
        nc.sync.dma_start(out=sq[:, :1024], in_=sq_d[:, :1024])
        nc.sync.dma_start(out=sq[:, 1024:], in_=sq_d[:, 1024:])
        nc.sync.dma_start(out=wv[:], in_=wv_d[:])
        nc.sync.dma_start(out=maskv[:], in_=mk_d[:])
        nc.sync.dma_start(out=wo[:], in_=wo_d[:])
        nc.vector.memset(onesc[:], 1.0)
        nc.vector.memset(onesr[:], 1.0)

        # ---- projections with fused RoPE, chunk-major over tokens ----
        def rope(pst, cos_sb, sin_sb, dst, sl):
            t1 = rtmp.tile([128, 512], F32, tag="t1")
            nc.vector.tensor_mul(t1[:], pst[:], cos_sb[:, sl])
            t2 = rtmp.tile([128, 512], F32, tag="t2")
            nc.vector.tensor_mul(t2[0:64, :], pst[64:128, :], sin_sb[0:64, sl])
            nc.vector.tensor_mul(t2[64:128, :], pst[0:64, :], sin_sb[64:128, sl])
            nc.vector.tensor_add(dst, t1[:], t2[:])

        for c in range(NTC):
            sl = slice(c * 512, (c + 1) * 512)
            xc = xpool.tile([128, NKB, 512], BF16, tag="xc")
            nc.sync.dma_start(out=xc[:], in_=xp_d[:, :, sl])
            # kT chunk
            pst = ps.tile([128, 512], F32, tag="ps")
            for j in range(NKB):
                nc.tensor.matmul(
                    pst[:],
                    lhsT=wk[:, j, :],
                    rhs=xc[:, j, :],
                    start=(j == 0),
                    stop=(j == NKB - 1),
                )
            rope(pst, ck, sk, kT[:, sl], sl)
            # qT chunks (4 heads)
            for h in range(NH):
                pst = ps.tile([128, 512], F32, tag="ps")
                for j in range(NKB):
                    nc.tensor.matmul(
                        pst[:],
                        lhsT=wq[:, j, h * 128 : (h + 1) * 128],
                        rhs=xc[:, j, :],
                        start=(j == 0),
                        stop=(j == NKB - 1),
                    )
                rope(pst, cq, sq, qT[:, h, sl], sl)
            # V blocks (tokens on partitions)
            for tb in range(4):
                pv = ps.tile([128, 512], F32, tag="ps")
                for j in range(NKB):
                    nc.tensor.matmul(
                        pv[:, :128],
                        lhsT=xc[:, j, tb * 128 : (tb + 1) * 128],
                        rhs=wv[:, j, :],
                        start=(j == 0),
                        stop=(j == NKB - 1),
                    )
                nc.scalar.copy(V[:, c * 4 + tb, :], pv[:, :128])

        # ---- attention + output projection, per q stripe ----
        LN = mybir.ActivationFunctionType.Ln
        for s in range(NTC):
            qsl = slice(s * 512, (s + 1) * 512)
            nb = 4 * (s + 1)
            OTs = otpool.tile([128, NH, 512], BF16, tag="OT")

            # finalize head h: broadcast 1/l across partitions (K=1 outer
            # product), then normalize O^T on the PSUM->SBUF move.  Emitted
            # a few PE instructions after the rl chain started so the PE
            # never waits on ACT/DVE latency.
            def finalize(h, rl, op):
                bc = ps.tile([128, 512], F32, tag="ps")
                nc.tensor.matmul(bc[:], lhsT=onesr[:], rhs=rl[:], start=True, stop=True)
                bcs = bcpool.tile([128, 512], F32, tag="bcs")
                nc.scalar.copy(bcs[:], bc[:])
                nc.vector.tensor_mul(OTs[:, h, :], op[:], bcs[:])

            pending = None
            for h in range(NH):
                PT = ptpool.tile([128, NTB, 512], BF16, tag="PT")
                lp = ps_l.tile([1, 512], F32, tag="lp")
                op = ps_o.tile([128, 512], F32, tag="op")

                def lmm(b):
                    nc.tensor.matmul(
                        lp[:],
                        lhsT=onesc[:],
                        rhs=PT[:, b, :],
                        start=(b == 0),
                        stop=(b == nb - 1),
                    )

                def pvmm(b):
                    nc.tensor.matmul(
                        op[:],
                        lhsT=V[:, b, :],
                        rhs=PT[:, b, :],
                        start=(b == 0),
                        stop=(b == nb - 1),
                    )

                for b in range(nb):
                    stp = ps.tile([128, 512], F32, tag="ps")
                    nc.tensor.matmul(
                        stp[:],
                        lhsT=kT[:, b * 128 : (b + 1) * 128],
                        rhs=qT[:, h, qsl],
                        start=True,
                        stop=True,
                    )
                    nc.scalar.activation(PT[:, b, :], stp[:], EXP)
                    if b >= 4 * s:
                        nc.vector.tensor_mul(
                            PT[:, b, :], PT[:, b, :], maskv[:, b - 4 * s, :]
                        )
                    if b == 3 and pending is not None:
                        finalize(*pending)
                        pending = None
                    if b >= 2:
                        lmm(b - 2)
                    if b >= 4:
                        pvmm(b - 4)
                lmm(nb - 2)
                lmm(nb - 1)
                # 1/l via exp(-ln(l)) on ACT: a [1,512] DVE reciprocal costs
                # ~4us (iterative), the two ACT LUT passes cost ~1us total.
                lnl = rlpool.tile([1, 512], F32, tag="lnl")
                nc.scalar.activation(lnl[:], lp[:], LN)
                rl = rlpool.tile([1, 512], BF16, tag="rl")
                nc.scalar.activation(rl[:], lnl[:], EXP, scale=-1.0)
                for b in range(max(nb - 4, 0), nb):
                    pvmm(b)
                if pending is not None:
                    finalize(*pending)
                pending = (h, rl, op)

            # Wo: y[q, :] = sum_h O_h[q, :] @ Wo_h; the last head's finalize
            # is slotted between the first Wo accumulation groups.
            for qb in range(4):
                ysb = ypool.tile([128, D], BF16, tag="y")
                for dc in range(4):
                    yp = ps_y.tile([128, 512], F32, tag="yp")
                    for h in range(NH - 1):
                        nc.tensor.matmul(
                            yp[:],
                            lhsT=OTs[:, h, qb * 128 : (qb + 1) * 128],
                            rhs=wo[:, h, dc * 512 : (dc + 1) * 512],
                            start=(h == 0),
                            stop=False,
                        )
                    if pending is not None:
                        finalize(*pending)
                        pending = None
                    h = NH - 1
                    nc.tensor.matmul(
                        yp[:],
                        lhsT=OTs[:, h, qb * 128 : (qb + 1) * 128],
                        rhs=wo[:, h, dc * 512 : (dc + 1) * 512],
                        start=False,
                        stop=True,
                    )
                    nc.vector.tensor_copy(ysb[:, dc * 512 : (dc + 1) * 512], yp[:])
                nc.sync.dma_start(
                    out=y_d[s * 512 + qb * 128 : s * 512 + (qb + 1) * 128, :],
                    in_=ysb[:],
                )

    nc.compile()
    _program = nc
    return nc


def _host_prep(x, Wq, Wk, Wv, Wo):
    x = np.asarray(x, dtype=np.float32)
    Wq = np.asarray(Wq, dtype=np.float32)
    Wk = np.asarray(Wk, dtype=np.float32)
    Wv = np.asarray(Wv, dtype=np.float32)
    Wo = np.asarray(Wo, dtype=np.float32)

    # RoPE even/odd gather folded into weight column permutation (per head)
    perm128 = np.r_[np.arange(0, 128, 2), np.arange(1, 128, 2)]
    permq = np.concatenate([hb * 128 + perm128 for hb in range(H)])
    permk = np.concatenate([hb * 128 + perm128 for hb in range(KV)])
    Wq_p = Wq[:, permq]
    Wk_p = Wk[:, permk]

    pos = np.arange(T, dtype=np.float64)
    inv_freq = 1.0 / (10000.0 ** (np.arange(0, HD, 2, dtype=np.float64) / HD))
    ang = np.einsum("t,f->tf", pos, inv_freq)  # [T, 64]
    cos = np.cos(ang).T.astype(np.float32)  # [64, T]
    sin = np.sin(ang).T.astype(np.float32)
    cosk = np.concatenate([cos, cos], axis=0)  # [128, T]
    sink = np.concatenate([-sin, sin], axis=0)
    cosq = (cosk * SCALE).astype(np.float32)
    sinq = (sink * SCALE).astype(np.float32)

    # binary causal masks for the 4 diagonal blocks of a 512-wide q stripe:
    # keep (kv_l <= q_l - 128*r) for relative kv block r
    kv_l = np.arange(128)[:, None]
    q_l = np.arange(512)[None, :]
    maskv = np.ascontiguousarray(
        np.stack([(kv_l <= q_l - 128 * r) for r in range(4)], axis=1)
    ).astype(bfloat16)  # [128, 4, 512]

    def pack_pj(w, cols):
        # [D, cols] -> [128, NKB, cols] with [p, j, c] = w[j*128+p, c]
        return np.ascontiguousarray(
            w.reshape(NKB, 128, cols).transpose(1, 0, 2)
        ).astype(bfloat16)

    in_maps = []
    for c in range(8):
        b, s = c // 4, c % 4
        xb = np.ascontiguousarray(x[b].T)  # [D, T]
        wo_sh = Wo[s * 512 : (s + 1) * 512, :]  # [512, D]
        in_maps.append(
            {
                "xp": pack_pj(xb, T),
                "Wq": pack_pj(Wq_p[:, s * 512 : (s + 1) * 512], NH * 128),
                "Wk": pack_pj(Wk_p[:, s * 128 : (s + 1) * 128], 128),
                "Wv": pack_pj(Wv[:, s * 128 : (s + 1) * 128], 128),
                "Wo": np.ascontiguousarray(
                    wo_sh.reshape(NH, 128, D).transpose(1, 0, 2)
                ).astype(bfloat16),
                "cosq": cosq,
                "sinq": sinq,
                "cosk": cosk,
                "sink": sink,
                "maskv": maskv,
            }
        )
    return in_maps


def _ensure_ntff_hook():
    """The agent image's antenv lacks axon_hooks, so boot() skips installing
    the NTFF profile hook. Recreate the module and install the hook."""
    import sys
    import types

    try:
        from antenv.axon_hooks import get_axon_ntff_profile_hook  # noqa: F401

        return True
    except ImportError:
        pass
    try:
        import antenv
        from trn_agent_boot.trn_boot import _ntff_profile_via_ctypes

        hook = _ntff_profile_via_ctypes("/opt/axon/libaxon_pjrt.so")
        if hook is None:
            return False
        mod = types.ModuleType("antenv.axon_hooks")
        mod._hook = hook
        mod.set_axon_ntff_profile_hook = lambda h: setattr(mod, "_hook", h)
        mod.get_axon_ntff_profile_hook = lambda: mod._hook
        sys.modules["antenv.axon_hooks"] = mod
        antenv.axon_hooks = mod
        bass_utils.upload_artifacts = lambda d: d
        return True
    except Exception:
        return False


def kernel(x, Wq, Wk, Wv, Wo):
    global _last_results, last_exec_time_ns
    nc = _build_program()
    in_maps = _host_prep(x, Wq, Wk, Wv, Wo)
    trace = bool(int(os.environ.get("KERNEL_TRACE", "0")))
    tmpdir = None
    if trace:
        trace = _ensure_ntff_hook()
        if trace:
            tmpdir = os.environ.get("KERNEL_TRACE_DIR") or None
    res = bass_utils.run_bass_kernel_spmd(
        nc, in_maps, core_ids=list(range(8)), trace=trace, tmpdir=tmpdir
    )
    _last_results = res
    last_exec_time_ns = res.exec_time_ns
    out = np.empty((B, T, D), dtype=np.float32)
    for b in range(B):
        out[b] = sum(
            res.results[4 * b + s]["y"].astype(np.float32) for s in range(TP)
        )
    return out
